# revision 39
# baseline (speedup 1.0000x reference)
"""Trainium2 Bass kernel for nn_MultiHeadAttention_83056077570808.

GQA multi-head attention (32 q heads, 8 kv heads, d_head=128, T=2048,
D=4096) with RoPE, tanh soft-capping at 30, causal mask, fp32 reference.

Sharding: tensor-parallel over heads across 8 cores. Core c owns kv head c
and q heads 4c..4c+3: Wq/Wk/Wv column-sharded, Wo row-sharded; activations
replicated. Each core computes a partial output (its heads' contribution
through its Wo rows); the host sums the 8 partials.

Fully streamed schedule: causality means attention chunk tcx only needs
K/V tiles 0..4*tcx+3, so K/V/Q projections for chunk tcx+1 run *during*
attention of chunk tcx as filler work woven between QK groups (covering
the ACT-engine tanh/exp latency); O-proj of chunk tcx-1 likewise. DMA is
spread across the whole timeline instead of front-loaded. Fillers are
paced by an explicit cost model (popping too fast blocks the in-order PE
stream on un-arrived slab DMAs; too slow starves PE under ACT).

Causal diagonal trim: for key tile Tt in the diagonal block of chunk tcx
(rel = Tt-4*tcx in 0..3), query columns < 128*rel are entirely masked, so
QK / tanh / exp are column-trimmed, only the [128,128] diagonal block is
tri-masked (Pool), and PV skips s4-blocks with s4 < rel.

Q/K/V and O projections run as fp8e4 DoubleRow matmuls (0.5 cycles/row,
2 stacked products per instruction) using a hi+lo residual 3-product
scheme (Xh@Wh + Xl@Wh + Xh@Wl at 0.75x the bf16 cycle cost, accuracy
slightly BETTER than bf16): W is scaled by W_SCALE=64 into fp8, X split
hi/lo at scale 1, with hi/lo interleaved per k-tile in DRAM so slab DMAs
stay 3-dim. The scales fold into existing constants (tanh scale /64^2,
PV ones-column = W_SCALE/A_SCALE, osb evac x1/(64*16)) at zero extra
ops; attnT is split hi/lo fp8 on device (one extra tensor_sub per
transpose evac). QK and PV stay bf16 (fp8 there busts the error budget).
PSUM accumulation fp32; rope arithmetic fp32.

PSUM bank rule in the PV accumulation: start=True clears has_written for
the WHOLE bank and two s-chains share each bank, so only the bank's first
chain issues start=True; the sibling chain's first write lands on cleared
bits and overwrites.
"""

import os
import sys

for _p in ("/opt/trn_rl_repo", os.path.expanduser("~/.axon_site/_ro/trn_rl_repo")):
    if os.path.isdir(_p) and _p not in sys.path:
        sys.path.insert(0, _p)

import numpy as np
import ml_dtypes

import concourse.bass as bass
import concourse.tile as tile
from concourse import bacc, mybir
from concourse.bass_utils import run_bass_kernel_spmd

F32 = mybir.dt.float32
BF16 = mybir.dt.bfloat16
FP8 = mybir.dt.float8e4
FP8NP = ml_dtypes.float8_e4m3
DR = mybir.MatmulPerfMode.DoubleRow
W_SCALE = 64.0  # fp8 weight scale; folded into tanh scale and vaug ones-col
A_SCALE = 16.0  # attn-output fp8 scale; folded into vaug ones-col + osb evac
OSB_SCALE = 1.0 / (W_SCALE * A_SCALE)

D_MODEL = 4096
KEY_SIZE = 128
NUM_Q_HEADS = 32
NUM_KV_HEADS = 8
N_CORES = 8
NH = NUM_Q_HEADS // NUM_KV_HEADS  # q heads per core = 4
ATTN_MULT = 0.08838834764831845
CAP = 30.0

Tanh = mybir.ActivationFunctionType.Tanh
Exp = mybir.ActivationFunctionType.Exp


PHASE_MARKS = []


def _mark(nc, label):
    n = nc.get_next_instruction_name()
    PHASE_MARKS.append((int(str(n).split("-")[-1]), label))


def build_nc(T: int, causal: bool):
    """Emit the Bass program for one core (SPMD: all cores run this).

    Tile builds a STATIC per-engine schedule in (priority = emission)
    order, so overlap must be engineered in the emission order itself.
    """
    D = D_MODEL
    TC = 512                 # t-chunk width
    NTC = T // TC            # t-chunks
    NTT = T // 128           # 128-tiles along T (key side)
    NDT = D // 128           # contraction tiles over d_model = 32
    JW = NH * KEY_SIZE       # per-core q/o width = 512
    GW = 2                   # key tiles per QK group (1 PSUM bank each)

    SPLIT_O = os.environ.get("MHA_SPLIT_O", "0") == "1"
    EVAC_ALT = os.environ.get("MHA_EVAC_ALT", "0") == "1"
    PROJ_ORDER2 = os.environ.get("MHA_PORDER2", "0") == "1"
    DRAIN2 = os.environ.get("MHA_DRAIN2", "0") == "1"
    DEFER_O = os.environ.get("MHA_DEFER_O", "0") == "1"
    KVSPILL = os.environ.get("MHA_KVSPILL", "1") == "1"
    QSPILL = os.environ.get("MHA_QSPILL", "0") == "1"
    PIPED = int(os.environ.get("MHA_PIPED", "5"))
    FINFILL = os.environ.get("MHA_FINFILL", "0") == "1"
    VLATE = os.environ.get("MHA_VLATE", "1") == "1"

    nc = bacc.Bacc(None, target_bir_lowering=False)

    xq = nc.dram_tensor("xq", [2 * D, T], FP8, kind="ExternalInput")
    xk = nc.dram_tensor("xk", [2 * D, T], FP8, kind="ExternalInput")
    xv = nc.dram_tensor("xv", [2 * D, T], FP8, kind="ExternalInput")
    wq = nc.dram_tensor("wq", [128, NH * NDT * 2 * 128], FP8, kind="ExternalInput")
    wk = nc.dram_tensor("wk", [128, NDT * 2 * 128], FP8, kind="ExternalInput")
    wv = nc.dram_tensor("wv", [128, NDT * 2 * 128], FP8, kind="ExternalInput")
    wo = nc.dram_tensor("wo", [128, NH * 2 * D], FP8, kind="ExternalInput")
    cosd = nc.dram_tensor("cosT", [128, T], BF16, kind="ExternalInput")
    sind = nc.dram_tensor("sinT", [128, T], BF16, kind="ExternalInput")
    rotd = nc.dram_tensor("rot", [128, 128], BF16, kind="ExternalInput")
    identbd = nc.dram_tensor("identb", [128, 128], BF16, kind="ExternalInput")
    trid = nc.dram_tensor("tri", [128, 128], BF16, kind="ExternalInput")
    outd = nc.dram_tensor("out", [T, D], BF16, kind="ExternalOutput")
    outa = nc.dram_tensor("outa", [TC, D], BF16, kind="ExternalOutput")
    outb = nc.dram_tensor("outb", [TC, D], BF16, kind="ExternalOutput")

    with tile.TileContext(nc) as tc:
        with (
            tc.tile_pool(name="const", bufs=1) as constp,
            tc.tile_pool(name="persist", bufs=1) as persist,
            tc.tile_pool(name="slabs", bufs=2) as slabp,
            tc.tile_pool(name="tmps", bufs=2) as tmpp,
            tc.tile_pool(name="pa", bufs=2) as pa,
            tc.tile_pool(name="qkps", bufs=2, space="PSUM") as qkps,
            tc.tile_pool(name="pvps", bufs=1, space="PSUM") as pvps,
            tc.tile_pool(name="mmps", bufs=2, space="PSUM") as mmps,
        ):
            # ---- persistent SBUF ----
            rot_sb = constp.tile([128, 128], BF16)
            identb_sb = constp.tile([128, 128], BF16)
            tri_sb = constp.tile([128, 128], BF16)
            cos_sb = constp.tile([128, T], BF16)
            sin_sb = constp.tile([128, T], BF16)
            kT_rope = persist.tile([128, T], BF16)
            vaug = persist.tile([128, NTT, 132], BF16)
            wq_sb = persist.tile([128, NH, NDT, 2, 128], FP8)
            wk_sb = persist.tile([128, NDT, 2, 128], FP8)
            wv_sb = persist.tile([128, NDT, 2, 128], FP8)
            wo_sb = persist.tile([128, NH, 2, D], FP8)

            # ---- (tiny const DMAs moved into the prologue, after the
            # critical wq-head-0 + first qslab transfers) ----
            # v carries the fp8 W_SCALE; the ones-column carries it too so
            # the normalize ratio cancels it.
            nc.any.memset(vaug[:, :, 128:132], W_SCALE / A_SCALE)

            # ---------------- emit-helper closures ----------------
            # Filler items are (cost_ns, fn) pairs.

            def kv_fillers(xsrc, w_sb, tch, dst_cb):
                """K or V projection of t-columns [tch*512,(tch+1)*512)."""
                st = {}

                def dma_i(i, half):
                    def f():
                        if half == 0:
                            st[i] = slabp.tile(
                                [128, 16, TC], FP8, tag="kvslab",
                                bufs=int(os.environ.get("MHA_KVBUFS", "3")),
                                name="kvslab",
                            )
                        g0 = i * 8 + half * 4
                        nc.sync.dma_start(
                            out=st[i][:, 8 * half : 8 * half + 8, :],
                            in_=xsrc[
                                g0 * 256 : (g0 + 4) * 256,
                                tch * TC : (tch + 1) * TC,
                            ].rearrange("(n k) t -> k n t", k=128),
                        )
                    return {"cost": 100, "fn": f, "dma": 1456, "kind": "dma",
                            "grp": "kv", "bar": tch}

                def comp_i(i):
                    # 3-product fp8 DoubleRow over 8 k-tiles: 4 hi@hi pair
                    # matmuls + 8 (w_lo,x_hi)+(w_hi,x_lo) corrections
                    def f():
                        if i == 0:
                            st["ps"] = mmps.tile(
                                [128, TC], F32, tag="mm", name="kv_ps"
                            )
                        ps = st["ps"]
                        for j in range(0, 8, 2):
                            nc.tensor.matmul(
                                ps,
                                w_sb[:, i * 8 + j : i * 8 + j + 2, 1, :],
                                st[i][:, 2 * j : 2 * j + 3 : 2, :],
                                start=(i == 0 and j == 0),
                                stop=False,
                                perf_mode=DR,
                            )
                        for j in range(8):
                            nc.tensor.matmul(
                                ps,
                                w_sb[:, i * 8 + j],
                                st[i][:, 2 * j : 2 * j + 2, :],
                                start=False,
                                stop=(i == 3 and j == 7),
                                perf_mode=DR,
                            )
                        if i == 3:
                            dst_cb(ps)
                    return {"cost": 1280, "fn": f, "dma": 0, "kind": "comp",
                            "grp": "kv", "bar": tch}

                return [dma_i(0, 0), dma_i(0, 1), dma_i(1, 0), comp_i(0),
                        dma_i(1, 1), dma_i(2, 0), comp_i(1), dma_i(2, 1),
                        dma_i(3, 0), comp_i(2), dma_i(3, 1), comp_i(3)]

            def rope(dst, src, t0, tw):
                """dst[128, tw] = RoPE(src[128, tw]) at positions t0.. (fp32
                math; src/dst bf16)."""
                rp = mmps.tile([128, TC], F32, tag="mm", name="rope_ps")
                nc.tensor.matmul(rp[:, :tw], rot_sb, src, start=True, stop=True)
                t1 = pa.tile([128, TC], F32, tag="rt1", bufs=1, name="rope_t1")
                nc.gpsimd.tensor_mul(t1[:, :tw], src, cos_sb[:, t0 : t0 + tw])
                t2 = pa.tile([128, TC], F32, tag="rt2", bufs=1, name="rope_t2")
                nc.vector.tensor_mul(t2[:, :tw], rp[:, :tw], sin_sb[:, t0 : t0 + tw])
                nc.vector.tensor_add(dst, t1[:, :tw], t2[:, :tw])

            def tag_stream(items, s):
                for it in items:
                    it["stream"] = s
                return items

            def k_chunk_fillers(tch):
                ktmp = tmpp.tile([128, TC], BF16, tag="ktmp", name="ktmp")

                def evac(ps):
                    nc.vector.tensor_copy(ktmp, ps)

                items = kv_fillers(xk, wk_sb, tch, evac)
                tag_stream(items, "k")

                def rope_k():
                    rope(kT_rope[:, tch * TC : (tch + 1) * TC], ktmp,
                         tch * TC, TC)

                return items + [{"cost": 350, "fn": rope_k, "dma": 0,
                                 "kind": "comp", "grp": "kv", "bar": tch,
                                 "stream": "k"}]

            def v_chunk_fillers(tch):
                vtmp = tmpp.tile([128, TC], BF16, tag="vtmp", name="vtmp")

                def evac(ps):
                    nc.vector.tensor_copy(vtmp, ps)

                items = kv_fillers(xv, wv_sb, tch, evac)
                tag_stream(items, "v")

                def vtr(half):
                    def f():
                        for b2 in range(2):
                            b = 4 * tch + 2 * half + b2
                            tp = mmps.tile(
                                [128, TC], BF16, tag="mm", name="vtr_ps"
                            )
                            nc.tensor.transpose(
                                tp[:, :128],
                                vtmp[:, (2 * half + b2) * 128 :
                                     (2 * half + b2 + 1) * 128],
                                identb_sb,
                            )
                            nc.vector.tensor_copy(vaug[:, b, 0:128], tp[:, :128])
                    return {"cost": 220, "fn": f, "dma": 0, "kind": "comp",
                            "grp": "kv", "bar": tch, "stream": "v"}

                return items + [vtr(0), vtr(1)]

            def qslab_dma_fillers(tcx):
                slabs = []

                def dma_h(dh, q):
                    def f():
                        if q == 0:
                            slab = slabp.tile(
                                [128, 32, TC], FP8, tag="qslab",
                                bufs=int(os.environ.get("MHA_QBUFS", "2")),
                                name="qslab",
                            )
                            slabs.append(slab)
                        slab = slabs[dh]
                        g0 = dh * 16 + q * 4
                        nc.sync.dma_start(
                            out=slab[:, 8 * q : 8 * q + 8, :],
                            in_=xq[
                                g0 * 256 : (g0 + 4) * 256,
                                tcx * TC : (tcx + 1) * TC,
                            ].rearrange("(n k) t -> k n t", k=128),
                        )
                    return {"cost": 100, "fn": f, "dma": 1456, "kind": "dma",
                            "grp": "pre"}

                return slabs, [dma_h(0, q) for q in range(4)] + [
                    dma_h(1, q) for q in range(4)
                ]

            def qproj_chain(slabs, qraw, jh):
                ps = mmps.tile([128, TC], F32, tag="mm", name="q_ps")
                for dh in range(2):
                    for i in range(0, 16, 2):
                        nc.tensor.matmul(
                            ps,
                            wq_sb[:, jh, dh * 16 + i : dh * 16 + i + 2, 1, :],
                            slabs[dh][:, 2 * i : 2 * i + 3 : 2, :],
                            start=(dh == 0 and i == 0),
                            stop=False,
                            perf_mode=DR,
                        )
                    for i in range(16):
                        nc.tensor.matmul(
                            ps,
                            wq_sb[:, jh, dh * 16 + i],
                            slabs[dh][:, 2 * i : 2 * i + 2, :],
                            start=False,
                            stop=(dh == 1 and i == 15),
                            perf_mode=DR,
                        )
                nc.vector.tensor_copy(qraw[:, jh, :], ps)

            def q_chunk_fillers(tcx, slabs):
                """Q proj + rope for chunk tcx; returns (qrope, items)."""
                qraw = tmpp.tile([128, NH, TC], BF16, tag="qraw", bufs=1, name="qraw")
                qrope = tmpp.tile([128, NH, TC], BF16, tag="qrope", bufs=2, name="qrope")
                items = []
                for jh in range(NH):
                    items.append({
                        "cost": 5120, "dma": 0, "kind": "comp", "grp": "q",
                        "bar": tcx, "qbar": jh,
                        "fn": lambda jh=jh: qproj_chain(slabs, qraw, jh),
                    })
                for jh in range(NH):
                    items.append({
                        "cost": 350, "dma": 0, "kind": "comp", "grp": "q",
                        "bar": tcx, "qbar": jh,
                        "fn": lambda jh=jh: rope(
                            qrope[:, jh, :], qraw[:, jh, :], tcx * TC, TC
                        ),
                    })
                return qrope, items

            def make_oproj_fillers(attnT, t0, jhs=range(NH), dest=None,
                                   dest_t0=None, evac_alt=False,
                                   use_qkps=False, batch2=False,
                                   tail=False):
                # batch2: pair s4 osb writes into one DMA — only safe in the
                # tail where no input reads queue behind the write's waits
                b2st = {}
                dest = outd if dest is None else dest
                dest_t0 = t0 if dest_t0 is None else dest_t0
                jhs = list(jhs)
                fillers = []
                ODL = int(os.environ.get("MHA_ODMA_LAST", "0"))
                for nch in range(D // TC):
                    for s4 in range(4):
                        def f(s4=s4, nch=nch):
                            with nc.named_scope("oproj"):
                                if use_qkps and (s4 + nch) % 2 == 0:
                                    # qk PSUM banks are idle in the tail:
                                    # alternate into them for a deeper
                                    # chain pipeline
                                    ps = qkps.tile(
                                        [128, GW, TC], F32, tag="qk",
                                        name="o_ps2",
                                    )[:, 0, :]
                                else:
                                    ps = mmps.tile(
                                        [128, TC], F32, tag="mm", name="o_ps"
                                    )
                                for x in range(0, len(jhs), 2):
                                    j0 = jhs[x]
                                    nc.tensor.matmul(
                                        ps,
                                        attnT[:, j0 : j0 + 2, 0,
                                              s4 * 128 : (s4 + 1) * 128],
                                        wo_sb[:, j0 : j0 + 2, 1,
                                              nch * TC : (nch + 1) * TC],
                                        start=(x == 0),
                                        stop=False,
                                        perf_mode=DR,
                                    )
                                for x, jh in enumerate(jhs):
                                    nc.tensor.matmul(
                                        ps,
                                        attnT[:, jh, :, s4 * 128 : (s4 + 1) * 128],
                                        wo_sb[:, jh, :, nch * TC : (nch + 1) * TC],
                                        start=False,
                                        stop=(x == len(jhs) - 1),
                                        perf_mode=DR,
                                    )
                                if batch2:
                                    if s4 % 2 == 0:
                                        b2st[nch] = pa.tile(
                                            [128, 2, TC], BF16, tag="osb2",
                                            bufs=2, name="osb2",
                                        )
                                    osb = b2st[nch][:, s4 % 2, :]
                                else:
                                    osb = pa.tile(
                                        [128, TC], BF16, tag="osb",
                                        bufs=int(os.environ.get(
                                            "MHA_OSBUFS", "6")),
                                        name="osb",
                                    )
                                on_act = evac_alt and (s4 + nch) % 2 == 0
                                if on_act:
                                    nc.scalar.activation(
                                        out=osb, in_=ps,
                                        func=mybir.ActivationFunctionType.Copy,
                                        scale=OSB_SCALE,
                                    )
                                else:
                                    nc.vector.tensor_scalar_mul(osb, ps, OSB_SCALE)
                                mode = os.environ.get("MHA_ODMA", "sync")
                                if tail and ODL and nch * 4 + s4 >= 32 - ODL:
                                    mode = "act"
                                if mode == "act":
                                    eng = nc.scalar
                                elif mode == "pool":
                                    eng = nc.gpsimd
                                else:
                                    eng = nc.sync
                                if batch2:
                                    if s4 % 2 == 1:
                                        eng.dma_start(
                                            out=dest[
                                                dest_t0 + (s4 - 1) * 128 :
                                                dest_t0 + (s4 + 1) * 128,
                                                nch * TC : (nch + 1) * TC,
                                            ].rearrange(
                                                "(s k) t -> k s t", k=128
                                            ),
                                            in_=b2st[nch],
                                        )
                                else:
                                    eng.dma_start(
                                        out=dest[
                                            dest_t0 + s4 * 128 :
                                            dest_t0 + (s4 + 1) * 128,
                                            nch * TC : (nch + 1) * TC,
                                        ],
                                        in_=osb,
                                    )
                        fillers.append(
                            {"cost": 160 * len(jhs), "fn": f,
                             "dma": 364, "kind": "oproj", "grp": "o"}
                        )
                return fillers

            def interleave(a, b):
                out = []
                ia = ib = 0
                na, nb = len(a), len(b)
                ILR = float(os.environ.get("MHA_ILR", "1.0"))
                while ia < na or ib < nb:
                    if ia * max(nb, 1) * ILR <= ib * max(na, 1) and ia < na:
                        out.append(a[ia]); ia += 1
                    elif ib < nb:
                        out.append(b[ib]); ib += 1
                    else:
                        out.append(a[ia]); ia += 1
                return out

            def proj_items_for(tcx):
                """All projection work for chunk tcx as a filler list, DMA
                items placed so transfers land just ahead of their use."""
                slabs_n, qdma = qslab_dma_fillers(tcx)
                kn = k_chunk_fillers(tcx)
                vn = v_chunk_fillers(tcx)
                qrope_n, qn = q_chunk_fillers(tcx, slabs_n)
                qpairs = [qn[0], qn[NH], qn[1], qn[NH + 1], qn[2],
                          qn[NH + 2], qn[3], qn[NH + 3]]
                if VLATE:
                    # v-stream last: its data isn't needed until group
                    # 2*tcx of the NEXT attention window (kv barrier), so
                    # keep the congested window's DMA queue for k/q
                    items = (
                        [kn[0], kn[1], qdma[0], qdma[1], kn[2], kn[3],
                         qdma[2], qdma[3], kn[4], kn[5], qdma[4], qdma[5],
                         kn[6], kn[7], qdma[6], qdma[7], kn[8], kn[9],
                         kn[10], kn[11], kn[12]]
                        + qpairs
                        + vn[:12] + [vn[12], vn[13]]
                    )
                else:
                    items = (
                        [kn[0], kn[1], qdma[0], qdma[1], kn[2], kn[3],
                         qdma[2], qdma[3], kn[4], kn[5], qdma[4], qdma[5],
                         kn[6], kn[7], qdma[6], qdma[7], kn[8], kn[9],
                         kn[10], kn[11], kn[12]]
                        + vn[:12] + [vn[12], vn[13]]
                        + qpairs
                    )
                return qrope_n, items

            # ---------------- chunk 0 prologue (inline, DMA-ordered) ----
            # Critical path to the first q chain: wq head 0 + both qslabs;
            # everything else (k/v slabs, cos/sin) streams behind and PE
            # picks it up between/after the q chains.
            def wq_head_dma(jh):
                nc.sync.dma_start(
                    out=wq_sb[:, jh, :, :],
                    in_=wq[:, jh * NDT * 2 * 128 : (jh + 1) * NDT * 2 * 128].rearrange(
                        "k (n two j) -> k n two j", two=2, j=128
                    ),
                )

            k0 = k_chunk_fillers(0)
            v0 = v_chunk_fillers(0)
            qslabs0, qdma0 = qslab_dma_fillers(0)
            # wq head 0 first half (tiles 0-15) is all the first q-chain
            # needs to begin
            nc.sync.dma_start(
                out=wq_sb[:, 0, : NDT // 2],
                in_=wq[:, : NDT * 128].rearrange(
                    "k (n two j) -> k n two j", two=2, j=128
                ),
            )
            qdma0[0]["fn"]()
            nc.sync.dma_start(
                out=wq_sb[:, 0, NDT // 2 :],
                in_=wq[:, NDT * 128 : NDT * 2 * 128].rearrange(
                    "k (n two j) -> k n two j", two=2, j=128
                ),
            )
            for it in qdma0[1:3]:
                it["fn"]()
            wq_head_dma(1)                     # chain 1 weights early
            for it in qdma0[3:5]:
                it["fn"]()
            nc.sync.dma_start(out=rot_sb, in_=rotd[:])
            nc.sync.dma_start(out=identb_sb, in_=identbd[:])
            nc.sync.dma_start(out=tri_sb, in_=trid[:])
            wq_head_dma(2)
            for it in qdma0[5:]:               # remaining quarter-slab dmas
                it["fn"]()
            nc.sync.dma_start(out=wk_sb, in_=wk.rearrange("k (n two j) -> k n two j", two=2, j=128))
            for it in k0[0:3]:                 # kslab dmas
                it["fn"]()
            wq_head_dma(3)
            qrope0, q0 = q_chunk_fillers(0, qslabs0)
            q0[0]["fn"](); q0[1]["fn"]()       # qproj chains 0,1
            k0[4]["fn"](); k0[5]["fn"]()       # kslab dmas
            q0[2]["fn"]()                      # qproj chain 2
            k0[3]["fn"]()                      # comp k piece 0
            nc.sync.dma_start(out=cos_sb[:, :TC], in_=cosd[:, :TC])
            nc.sync.dma_start(out=sin_sb[:, :TC], in_=sind[:, :TC])
            q0[3]["fn"]()                      # qproj chain 3
            nc.sync.dma_start(out=cos_sb[:, TC:], in_=cosd[:, TC:])
            nc.sync.dma_start(out=sin_sb[:, TC:], in_=sind[:, TC:])
            k0[7]["fn"](); k0[8]["fn"](); k0[10]["fn"]()   # kslab dmas
            k0[6]["fn"](); k0[9]["fn"](); k0[11]["fn"]()   # comp k 1-3 + evac
            nc.sync.dma_start(out=wv_sb, in_=wv.rearrange("k (n two j) -> k n two j", two=2, j=128))
            k0[12]["fn"]()                     # rope-k(0)
            for it in q0[NH:]:                 # 4 rope-q(0)
                it["fn"]()
            if os.environ.get("MHA_V0SLIDE", "0") == "1":
                # v(0) isn't needed until head 0's PV drain in window 0:
                # run it as window-0 filler work instead of serializing the
                # DMA-congested prologue on it.
                v0_carry = v0
                for it in v0_carry:
                    it["grp"] = "v0"
            else:
                v0_carry = []
                v0[0]["fn"](); v0[1]["fn"](); v0[2]["fn"]()    # vslab dmas
                v0[3]["fn"]()                      # comp v piece 0
                v0[4]["fn"](); v0[5]["fn"]()       # vslab dmas
                v0[6]["fn"]()                      # comp v piece 1
                v0[7]["fn"](); v0[8]["fn"]()       # vslab dmas
                v0[9]["fn"]()                      # comp v piece 2
                v0[10]["fn"]()                     # vslab dma
                v0[11]["fn"]()                     # comp v piece 3 + evac
                v0[12]["fn"](); v0[13]["fn"]()     # vtr halves

            def wo_slice_dma(nch):
                def f():
                    nc.sync.dma_start(
                        out=wo_sb[:, :, :, nch * TC : (nch + 1) * TC],
                        in_=wo.rearrange("k (h two d) -> k h two d", two=2, d=D)[
                            :, :, :, nch * TC : (nch + 1) * TC
                        ],
                    )
                return {"cost": 100, "fn": f, "dma": 1456, "kind": "dma",
                        "grp": "pre"}

            # ---------------- main loop over t-chunks ----------------
            qrope_cur = qrope0
            prev_attnT = None
            prev_t0 = 0
            carry = []          # deferred oproj fillers from chunk tcx-1
            pend_carry = []     # attn groups of chunk tcx+1 head 0 emitted
                                # early (QK+act done at the boundary drain)
            kv_carry = []       # K/V-proj fillers spilled into their own
                                # attention window (barrier at group 2*tcx)
            for tcx in range(NTC):
                _mark(nc, f"window{tcx}")
                t0 = tcx * TC

                if tcx + 1 < NTC:
                    qrope_next, proj_items = proj_items_for(tcx + 1)
                else:
                    qrope_next, proj_items = None, []
                # wo: first 2 slices during attn(0) (needed by the first
                # oproj pops early in attn(1)), the rest during attn(1)
                # where the DMA queue has slack.
                WOS1 = int(os.environ.get("MHA_WOS1", "8"))
                if tcx == 0:
                    wos = [wo_slice_dma(n) for n in range(D // TC)]
                    proj_items = interleave(proj_items, wos[:2])
                elif tcx == 1:
                    proj_items = interleave(proj_items, wos[2:WOS1])
                elif tcx == 2 and WOS1 < D // TC:
                    proj_items = interleave(proj_items, wos[WOS1:])
                oproj_items = carry + (
                    make_oproj_fillers(prev_attnT, prev_t0)
                    if prev_attnT is not None
                    else []
                )
                if DEFER_O:
                    if tcx == 1:
                        deferred_o = oproj_items
                        oproj_items = []
                    elif tcx == 2:
                        oproj_items = deferred_o + oproj_items
                if tcx == 0:
                    kv_carry = v0_carry + kv_carry
                NOIL = os.environ.get("MHA_NOIL", "")
                if str(tcx) in NOIL.split(","):
                    # PE-rich window: run proj first, let o-fillers slide
                    # into the next (ACT-bound) window via carry
                    fillers = kv_carry + proj_items + oproj_items
                else:
                    fillers = kv_carry + interleave(proj_items, oproj_items)
                kv_carry = []
                # annotate each compute item with the cumulative input-DMA
                # time that precedes it in this window's queue — popping it
                # earlier than that would head-of-line block the in-order
                # PE stream on an un-arrived transfer.
                cum_dma = 0.0
                for it in fillers:
                    if it["kind"] == "dma":
                        cum_dma += it["dma"]
                    if it["kind"] == "comp":
                        it["ready"] = cum_dma
                    elif it["kind"] == "oproj" and tcx == 1:
                        # wo slices still streaming in this window
                        it["ready"] = cum_dma
                    else:
                        it["ready"] = 0.0

                nt_valid = 4 * (tcx + 1) if causal else NTT
                ngroups = nt_valid // GW
                attnT = pa.tile(
                    [128, NH, 2, TC], FP8, tag="attnT", bufs=3, name="attnT"
                )
                budget = 0.0
                popped = 0.0
                qkpv_clock = 0.0
                act_clock = 0.0
                popped_dma = 0.0
                SLACK = float(os.environ.get("MHA_SLACK", "7000"))
                LOOKAHEAD = float(os.environ.get("MHA_LOOKAHEAD", "15000"))
                BMULT = float(os.environ.get("MHA_BMULT", "1.3"))

                DRIFT = float(os.environ.get("MHA_DRIFT", "1.0"))

                def pop_fillers():
                    nonlocal popped, popped_dma
                    while popped < budget and fillers:
                        elapsed = max(act_clock, qkpv_clock + popped) * DRIFT
                        # pull any leading dma items (keep the queue fed,
                        # but no more than LOOKAHEAD ahead of real time)
                        i = 0
                        progress = False
                        while i < len(fillers):
                            it = fillers[i]
                            if (it["kind"] == "dma"
                                    and popped_dma < elapsed + LOOKAHEAD):
                                fillers.pop(i)
                                it["fn"]()
                                popped_dma += it["dma"]
                                progress = True
                                continue
                            if it["kind"] != "dma":
                                break
                            i += 1
                        if not fillers or popped >= budget:
                            break
                        head = fillers[0]
                        if (head["kind"] != "dma"
                                and head["ready"] <= elapsed + SLACK):
                            fillers.pop(0)
                            head["fn"]()
                            popped += head["cost"]
                            popped_dma += head["dma"]
                            progress = True
                        elif head["kind"] != "dma":
                            # head blocked: pop a later independent item
                            # (oproj / q are reorderable; kv chains are not)
                            for j in range(1, min(len(fillers), 32)):
                                itj = fillers[j]
                                if (itj["kind"] != "dma"
                                        and itj.get("grp") in ("o", "q")
                                        and itj["ready"] <= elapsed + SLACK):
                                    fillers.pop(j)
                                    itj["fn"]()
                                    popped += itj["cost"]
                                    popped_dma += itj["dma"]
                                    progress = True
                                    break
                        if not progress:
                            break
                for h in range(NH):
                    if tcx >= 1:
                        i = 0
                        while i < len(fillers):
                            it = fillers[i]
                            if (it.get("grp") == "q" and it.get("bar") == tcx
                                    and it.get("qbar", 9) <= h):
                                fillers.pop(i)
                                it["fn"]()
                                popped += it["cost"]
                                popped_dma += it["dma"]
                            else:
                                i += 1
                    if h == 0 and pend_carry:
                        pend = pend_carry
                        pend_carry = []
                        g0 = len(pend)
                    else:
                        pend = []
                        g0 = 0
                    with nc.named_scope("attn"):
                        pv = pvps.tile(
                            [128, 4, 256], F32, tag="pv", name="pv_ps"
                        )
                        for gg in range(g0, ngroups):
                            VDEFER = os.environ.get("MHA_VDEFER", "0") == "1"
                            kbar = 2 * tcx if causal else 0
                            vbar = min(kbar + PIPED, ngroups - 1) if VDEFER else kbar
                            if h == 0 and tcx >= 1 and gg in (kbar, vbar):
                                # force-drain this chunk's spilled K/V work:
                                # the next QK group reads the new tiles.
                                # Issue the barrier's DMAs first, then weave
                                # independent o/q fillers between the kv
                                # comps so PE isn't head-of-line blocked on
                                # transfers still in flight.
                                bar_dmas, bar_comps, i = [], [], 0
                                while i < len(fillers):
                                    it = fillers[i]
                                    if (it.get("bar") == tcx
                                            and (gg == vbar
                                                 or it.get("stream") != "v")):
                                        fillers.pop(i)
                                        (bar_dmas if it["kind"] == "dma"
                                         else bar_comps).append(it)
                                    else:
                                        i += 1
                                for it in bar_dmas:
                                    it["fn"]()
                                    popped_dma += it["dma"]
                                for it in bar_comps:
                                    j = 0
                                    while j < len(fillers):
                                        itj = fillers[j]
                                        if (itj["kind"] != "dma"
                                                and itj.get("grp") == "o"):
                                            fillers.pop(j)
                                            itj["fn"]()
                                            popped += itj["cost"]
                                            popped_dma += itj["dma"]
                                            break
                                        j += 1
                                    it["fn"]()
                                    popped += it["cost"]
                                    popped_dma += it["dma"]
                            qk = qkps.tile(
                                [128, GW, TC], F32, tag="qk", name="qk_ps"
                            )
                            rels = []
                            for b in range(GW):
                                Tt = GW * gg + b
                                rel = Tt - 4 * tcx if causal else -1
                                rels.append(rel)
                                c0 = 128 * rel if rel > 0 else 0
                                nc.tensor.matmul(
                                    qk[:, b, c0:TC],
                                    kT_rope[:, Tt * 128 : (Tt + 1) * 128],
                                    qrope_cur[:, h, c0:TC],
                                    start=True,
                                    stop=True,
                                )
                            # tanh in place in PSUM, then exp to bf16 SBUF;
                            # soft-capping scales fused into ACT. Columns
                            # below the causal diagonal are skipped.
                            pt = pa.tile(
                                [128, GW, TC], BF16, tag="pt", bufs=int(os.environ.get("MHA_PTBUFS", "6")),
                                name="ptile",
                            )
                            act_cols = 0
                            if max(rels) <= 0:
                                nc.scalar.activation(
                                    out=qk, in_=qk, func=Tanh,
                                    scale=ATTN_MULT / CAP / (W_SCALE * W_SCALE),
                                )
                                nc.scalar.activation(
                                    out=pt, in_=qk, func=Exp, scale=CAP
                                )
                                act_cols = GW * TC
                            else:
                                for b in range(GW):
                                    c0 = 128 * max(rels[b], 0)
                                    nc.scalar.activation(
                                        out=qk[:, b, c0:TC],
                                        in_=qk[:, b, c0:TC],
                                        func=Tanh, scale=ATTN_MULT / CAP / (W_SCALE * W_SCALE),
                                    )
                                    nc.scalar.activation(
                                        out=pt[:, b, c0:TC],
                                        in_=qk[:, b, c0:TC],
                                        func=Exp, scale=CAP,
                                    )
                                    act_cols += TC - c0
                            for b in range(GW):
                                rel = rels[b]
                                if 0 <= rel < 4:
                                    # triangular mask on the diagonal block
                                    nc.gpsimd.tensor_mul(
                                        pt[:, b, rel * 128 : (rel + 1) * 128],
                                        pt[:, b, rel * 128 : (rel + 1) * 128],
                                        tri_sb,
                                    )
                            # software-pipelined PV: emit the PREVIOUS
                            # group's PV now, so it reaches PE well after
                            # its exp() finished on ACT (the current QK +
                            # fillers cover the ACT latency).
                            def emit_pv(p_pt, p_rels, p_gg):
                                n_pv = 0
                                for s4 in range(4):
                                    for b in range(GW):
                                        Tt = GW * p_gg + b
                                        rel = p_rels[b]
                                        if causal and rel > s4:
                                            continue
                                        n_pv += 1
                                        nc.tensor.matmul(
                                            pv[:, s4, 0:129],
                                            p_pt[:, b, s4 * 128 : (s4 + 1) * 128],
                                            vaug[:, Tt, 0:129],
                                            start=(
                                                p_gg == 0 and b == 0
                                                and s4 % 2 == 0
                                            ),
                                            stop=(
                                                (Tt == 4 * tcx + s4)
                                                if causal
                                                else (p_gg == ngroups - 1
                                                      and b == GW - 1)
                                            ),
                                            skip_group_check=True,
                                        )
                                return n_pv

                            n_pv = 0
                            pend.append((pt, rels, gg))
                            if len(pend) > PIPED:
                                n_pv = emit_pv(*pend.pop(0))
                            # weave fillers so PE stays busy under ACT
                            act_ns = act_cols * 2 * 0.833 + (
                                330 if max(rels) <= 0 else 660
                            )
                            qkpv_ns = (act_cols + 129 * n_pv) * 0.4167
                            act_clock += act_ns
                            qkpv_clock += qkpv_ns
                            budget += BMULT * max(act_ns - qkpv_ns, 0.0)
                            pop_fillers()
                        if tcx >= 1 and h == 0:
                            i = 0
                            while i < len(fillers):
                                if fillers[i].get("bar") == tcx:
                                    it = fillers.pop(i)
                                    it["fn"]()
                                    popped += it["cost"]
                                    popped_dma += it["dma"]
                                else:
                                    i += 1
                        if tcx == 0:
                            i = 0
                            while i < len(fillers):
                                if fillers[i].get("grp") == "v0":
                                    it = fillers.pop(i)
                                    it["fn"]()
                                    popped += it["cost"]
                                    popped_dma += it["dma"]
                                else:
                                    i += 1
                        while pend:
                            emit_pv(*pend.pop(0))
                    with nc.named_scope("attn_fin"):
                        ans = []
                        for s4 in range(4):
                            rc = pa.tile(
                                [128, 1], F32, tag="rc", bufs=4, name="rc"
                            )
                            nc.vector.reciprocal(rc, pv[:, s4, 128:129])
                            an = pa.tile(
                                [128, 128], BF16, tag="an", bufs=4, name="an"
                            )
                            nc.vector.tensor_scalar_mul(an, pv[:, s4, 0:128], rc)
                            ans.append(an)
                        # cover the DVE normalize latency with a filler
                        FINB = float(os.environ.get("MHA_FINB", "700"))
                        budget += FINB
                        act_clock += FINB
                        pop_fillers()
                        if FINFILL:
                            # transposes aren't needed until next chunk's
                            # O-proj: queue them as fillers instead of
                            # serializing at the head boundary
                            def fin_tr(ans=ans, h=h):
                                for s4 in range(4):
                                    tp = mmps.tile(
                                        [128, TC], BF16, tag="mm", name="atr"
                                    )
                                    nc.tensor.transpose(
                                        tp[:, :128], ans[s4], identb_sb
                                    )
                                    nc.vector.tensor_copy(
                                        attnT[:, h, 0, s4 * 128 : (s4 + 1) * 128],
                                        tp[:, :128],
                                    )
                                    nc.vector.tensor_sub(
                                        attnT[:, h, 1, s4 * 128 : (s4 + 1) * 128],
                                        tp[:, :128],
                                        attnT[:, h, 0, s4 * 128 : (s4 + 1) * 128],
                                    )
                            fillers.insert(0, {
                                "cost": 900, "fn": fin_tr, "dma": 0,
                                "kind": "oproj", "grp": "pre", "ready": 0.0,
                            })
                        else:
                            for s4 in range(4):
                                tp = mmps.tile(
                                    [128, TC], BF16, tag="mm", name="atr"
                                )
                                nc.tensor.transpose(
                                    tp[:, :128], ans[s4], identb_sb
                                )
                                nc.vector.tensor_copy(
                                    attnT[:, h, 0, s4 * 128 : (s4 + 1) * 128],
                                    tp[:, :128],
                                )
                                nc.vector.tensor_sub(
                                    attnT[:, h, 1, s4 * 128 : (s4 + 1) * 128],
                                    tp[:, :128],
                                    attnT[:, h, 0, s4 * 128 : (s4 + 1) * 128],
                                )
                    if SPLIT_O and tcx == NTC - 1 and h == 1:
                        for it in make_oproj_fillers(
                            attnT, t0, jhs=[0, 1], dest=outa, dest_t0=0
                        ):
                            it["ready"] = 0.0
                            fillers.append(it)
                # drain: 'pre' items (q proj/rope of tc+1) must finish
                # before attn(tcx+1) emits its first QK; K/V items of tc+1
                # spill into attn(tcx+1) (barrier at group 2*(tcx+1)), and
                # up to MHA_CARRY oproj items carry over (attnT bufs=3).
                carry = []
                rest = fillers
                if tcx + 1 < NTC:
                    cap = int(os.environ.get("MHA_CARRY", "24"))
                    o_total = sum(1 for it in rest if it["grp"] == "o")
                    drain_o = max(0, o_total - cap)
                    drain = []
                    for it in rest:
                        if it["grp"] == "pre":
                            drain.append(it)
                        elif it["grp"] == "q":
                            if it["qbar"] == 0 or not QSPILL:
                                drain.append(it)
                            else:
                                kv_carry.append(it)
                        elif it["grp"] == "o" and drain_o > 0:
                            drain.append(it)
                            drain_o -= 1
                        elif it["grp"] == "kv" and KVSPILL:
                            kv_carry.append(it)
                        elif it["grp"] == "kv":
                            drain.append(it)
                        else:
                            carry.append(it)
                    rest = drain
                # drain with the same dma-forwarding discipline: keep
                # transfers ~LOOKAHEAD ahead of the estimated PE clock so
                # in-order compute items rarely wait on arrival.
                if not DRAIN2:
                    for it in rest:
                        it["fn"]()
                    rest = []
                el = max(act_clock, qkpv_clock + popped)
                dma_el = popped_dma
                while rest:
                    i = 0
                    while i < len(rest):
                        if (rest[i]["kind"] == "dma"
                                and dma_el < el + LOOKAHEAD):
                            it = rest.pop(i)
                            it["fn"]()
                            dma_el += it["dma"]
                            continue
                        if rest[i]["kind"] != "dma":
                            break
                        i += 1
                    if not rest:
                        break
                    it = rest.pop(0)
                    it["fn"]()
                    el = max(el, it.get("ready", 0.0)) + it["cost"]
                    dma_el += it["dma"]
                EARLY = int(os.environ.get("MHA_EARLY", "0"))
                if causal and tcx + 1 < NTC and EARLY > 0:
                    # emit the first EARLY QK+act groups of the NEXT chunk's
                    # head 0 now: their tanh/exp runs on ACT while PE churns
                    # the boundary q-chain drain, pre-warming the next
                    # window's pend pipeline (relieves the ACT-bound tail
                    # windows). Tiles 0..2*EARLY-1 are chunk-0 K tiles, so
                    # no causal trim and no kv(tcx+1) dependency.
                    tnx = tcx + 1
                    with nc.named_scope("attn_early"):
                        for gg in range(EARLY):
                            qk = qkps.tile(
                                [128, GW, TC], F32, tag="qk", name="qk_ps"
                            )
                            rels = [GW * gg + b - 4 * tnx for b in range(GW)]
                            for b in range(GW):
                                Tt = GW * gg + b
                                nc.tensor.matmul(
                                    qk[:, b, :],
                                    kT_rope[:, Tt * 128 : (Tt + 1) * 128],
                                    qrope_next[:, 0, :],
                                    start=True,
                                    stop=True,
                                )
                            pt = pa.tile(
                                [128, GW, TC], BF16,
                                tag="pt",
                                bufs=int(os.environ.get("MHA_PTBUFS", "6")),
                                name="ptile",
                            )
                            nc.scalar.activation(
                                out=qk, in_=qk, func=Tanh,
                                scale=ATTN_MULT / CAP / (W_SCALE * W_SCALE),
                            )
                            nc.scalar.activation(
                                out=pt, in_=qk, func=Exp, scale=CAP
                            )
                            pend_carry.append((pt, rels, gg))
                qrope_cur = qrope_next
                prev_attnT, prev_t0 = attnT, t0

            _mark(nc, "tail")
            # tail: O proj pass B of the last chunk (host adds outa+outb)
            tail_items = (
                make_oproj_fillers(prev_attnT, prev_t0, jhs=[2, 3],
                                   dest=outb, dest_t0=0, evac_alt=True)
                if SPLIT_O
                else make_oproj_fillers(prev_attnT, prev_t0, evac_alt=True,
                                        use_qkps=True,
                                        batch2=os.environ.get(
                                            "MHA_TBATCH2", "0") == "1",
                                        tail=True)
            )
            for it in carry + tail_items:
                it["fn"]()

    nc.compile()
    return nc


def vbgd_dst(vaug):
    return vaug[:, :, 128:132]


def _host_constants(T: int):
    d = KEY_SIZE
    inv_freq = 1.0 / (10000.0 ** (np.arange(0, d, 2, dtype=np.float64) / d))  # [64]
    pos = np.arange(T, dtype=np.float64)
    phase_half = pos[None, :] * inv_freq[:, None]  # [64, T]
    phase = np.concatenate([phase_half, phase_half], axis=0)  # [128, T] (tiled)
    cosT = np.cos(phase).astype(np.float32)
    sinT = np.sin(phase).astype(np.float32)

    R = np.zeros((128, 128), dtype=np.float32)
    R[:64, 64:] = -np.eye(64, dtype=np.float32)
    R[64:, :64] = np.eye(64, dtype=np.float32)
    rot = np.ascontiguousarray(R.T)

    ident = np.eye(128, dtype=np.float32)

    # tri[k, c] = 1 if k <= c (valid: query col >= key row inside the
    # diagonal 128x128 block)
    tri = (np.arange(128)[:, None] <= np.arange(128)[None, :]).astype(
        ml_dtypes.bfloat16
    )

    NTT = T // 128
    vbg = np.zeros((128, NTT, 4), dtype=ml_dtypes.bfloat16)
    vbg[:, :, 0] = 1.0
    return cosT, sinT, rot, ident, tri, vbg


_NC_CACHE: dict = {}
LAST_RESULT = None
_LAST_IN_MAPS = None


def kernel(query, key, value, mask, Wq, Wk, Wv, Wo):
    global LAST_RESULT, _LAST_IN_MAPS
    query = np.asarray(query)
    key = np.asarray(key)
    value = np.asarray(value)
    mask = np.asarray(mask)
    Wq = np.asarray(Wq, dtype=np.float32)
    Wk = np.asarray(Wk, dtype=np.float32)
    Wv = np.asarray(Wv, dtype=np.float32)
    Wo = np.asarray(Wo, dtype=np.float32)

    b, T, D = query.shape
    assert b == 1 and D == D_MODEL, (b, D)

    m2 = np.asarray(mask).reshape(T, T).astype(bool)
    if np.array_equal(m2, np.tril(np.ones((T, T), dtype=bool))):
        causal = True
    elif m2.all():
        causal = False
    else:
        raise ValueError("unsupported mask pattern (expected causal or full)")

    kkey = (T, causal)
    if kkey not in _NC_CACHE:
        _NC_CACHE[kkey] = build_nc(T, causal)
    nc = _NC_CACHE[kkey]

    pnp = ml_dtypes.bfloat16

    def pack_x(x):
        # [T, D] fp32 -> [2*D, T] fp8: per k-tile 128-row blocks of hi then
        # lo, interleaved (tile, 2, 128, T)
        xT = np.ascontiguousarray(x.T, dtype=np.float32)
        hi = xT.astype(FP8NP)
        lo = (xT - hi.astype(np.float32)).astype(FP8NP)
        ndt = xT.shape[0] // 128
        a = np.stack([hi.reshape(ndt, 128, -1), lo.reshape(ndt, 128, -1)], axis=1)
        return np.ascontiguousarray(a.reshape(2 * xT.shape[0], xT.shape[1]))

    xq = pack_x(query[0])
    xk = pack_x(key[0])
    xv = pack_x(value[0])
    cosT, sinT, rot, ident, tri, vbg = _host_constants(T)

    JW = NH * KEY_SIZE
    NDT = D // 128

    def pack_wo(w):
        # [JW, D] -> [128, NH, 2, D] fp8 (lo, hi) per head, flattened
        ws = np.ascontiguousarray(w, dtype=np.float32) * W_SCALE
        hi = ws.astype(FP8NP)
        lo = (ws - hi.astype(np.float32)).astype(FP8NP)
        hi = hi.reshape(NH, 128, D_MODEL).transpose(1, 0, 2)
        lo = lo.reshape(NH, 128, D_MODEL).transpose(1, 0, 2)
        a = np.stack([lo, hi], axis=2)  # [128, NH, 2, D]
        return np.ascontiguousarray(a.reshape(128, NH * 2 * D_MODEL))

    def pack_w(w, nh):
        # [D, nh*128] -> [128, nh, NDT, 2, 128] fp8 (lo, hi) per k-tile,
        # flattened. Scaled by W_SCALE (folded out later on device).
        ws = np.ascontiguousarray(w, dtype=np.float32) * W_SCALE
        hi = ws.astype(FP8NP)
        lo = (ws - hi.astype(np.float32)).astype(FP8NP)
        # [D, nh*128] -> [NDT, 128k, nh, 128j] -> [128k, nh, NDT, 128j]
        hi = hi.reshape(NDT, 128, nh, 128).transpose(1, 2, 0, 3)
        lo = lo.reshape(NDT, 128, nh, 128).transpose(1, 2, 0, 3)
        a = np.stack([lo, hi], axis=3)  # [128, nh, NDT, 2, 128]
        return np.ascontiguousarray(a.reshape(128, nh * NDT * 2 * 128))

    in_maps = []
    for c in range(N_CORES):
        in_maps.append(
            {
                "xq": xq,
                "xk": xk,
                "xv": xv,
                "wq": pack_w(Wq[:, c * JW : (c + 1) * JW], NH),
                "wk": pack_w(Wk[:, c * KEY_SIZE : (c + 1) * KEY_SIZE], 1),
                "wv": pack_w(Wv[:, c * KEY_SIZE : (c + 1) * KEY_SIZE], 1),
                "wo": pack_wo(Wo[c * JW : (c + 1) * JW, :]),
                "cosT": cosT.astype(pnp),
                "sinT": sinT.astype(pnp),
                "rot": rot.astype(pnp),
                "identb": ident.astype(pnp),
                "tri": tri,
            }
        )

    _LAST_IN_MAPS = in_maps
    trace = os.environ.get("MHA_TRACE") == "1"
    res = run_bass_kernel_spmd(nc, in_maps, list(range(N_CORES)), trace=trace)
    LAST_RESULT = res

    out = np.zeros((T, D), dtype=np.float64)
    for c in range(N_CORES):
        out += res.results[c]["out"].astype(np.float64)
    return out.astype(np.float32).reshape(1, T, D)

